# revision 3
# baseline (speedup 1.0000x reference)
"""Trainium2 Bass kernel for nn_MultiHeadAttention (B=2, S=4096, D=512, H=8).

Sharding: sequence-parallel over queries. 8 cores = 2 batches x 4 query
quarters of 1024 rows each. Each core holds the full (mask-compacted) K/V
of its batch, computes its query rows end-to-end (Q/K/V projections,
masked softmax attention, output projection), and writes its disjoint
output rows. Host concatenates - no collectives needed.

Mask handling: the mask is a key-padding mask (per batch, per key).
Masked keys contribute exactly zero to softmax numerator and denominator,
so we compact them away on the host (halves all attention work; the
result is mathematically identical). Padding rows up to a multiple of
128 get a -30 additive bias so exp() sends them to ~1e-13.

Performance structure (v2): the kernel is jointly limited by the PE
(matmul fills) and ScalarE (exp activations, 1 elem/cycle/lane). The
projections are software-pipelined INTO the attention loop so the exp
stream starts ~15us into the kernel instead of after all projections
(~67us), and the output projection consumes the tail. A fraction of the
exp tiles are computed on the Vector engine via a bf16 Schraudolph
approximation (int16(A*logit+B) bit-cast to bf16), which offloads the
saturated ScalarE; the fraction is capped so the ~1.8% RMS approx noise
stays well inside the 2e-2 correctness budget.

Device dataflow (per core, SQ=1024 query rows, SK ~= 2176 keys):
  QT[o,q]  = WqT.T @ xqT     (bf16 matmuls, fp32 PSUM accumulation)
  KT[o,k]  = WkT.T @ xkT
  V[k,o]   = xvT.T @ WvT     -> packed as Vpad[k][h][V_h(64) | ones(64)]
  per head pair hc, key block kc:
    S^T[k,q] = KT_h.T @ QT_h          (two heads concurrent on PE row
                                       groups 0:63 / 64:127)
    E[k,q]   = exp(0.125*S^T + bias[k])   ScalarE exp -> bf16, or
               bitcast_bf16(int16(A*(0.125*S^T) + bias2[k]))  on DVE
    PV[128,q] accum= Vpad_h.T @ E     rows 0:64 = numerator^T,
                                      rows 64:128 = denominator (x64)
  OnT[h] = numerator^T * 1/denominator  (DVE recip + GpSimd mult, bf16)
  out[q,j] = sum_h OnT_h.T @ WoT_h  (fp32) -> DMA to DRAM
"""

import numpy as np
import ml_dtypes

B, S, D, H, DK = 2, 4096, 512, 8, 64
NCORES = 8
QSH = 4          # query shards per batch
SQ = S // QSH    # 1024 query rows per core

BF16 = ml_dtypes.bfloat16

# bf16 Schraudolph exp: exp(L) ~= bitcast_bf16(int16(SCH_A*L + SCH_B))
# (int16 truncation; SCH_B tuned numerically for min RMS rel err ~1.8%)
SCH_A = 128.0 / np.log(2.0)          # 184.6650
SCH_B = 127.0 * 128.0 - 7.0          # 16249.0
# Which (kc, hp=1) exp tiles run on the DVE instead of ScalarE.
# 11/17 of hp1 tiles -> 44/136 total: iterations where both exp tiles
# run on ScalarE cost 2.23us > the ~2.0us PE pace, so most iterations
# split hp0->ScalarE / hp1->DVE to run both exps concurrently. (This
# share only became profitable once the PV software-pipeline lag
# decoupled the PE from the DVE queue's latency; an even-kc alignment
# variant measured worse.) The approx noise adds ~1.0% error on a
# ~0.5% bf16 base - well inside the 2e-2 gate.
def _dve_kc(kc):
    return kc % 3 != 0

_BUILD_CACHE = {}
LAST_RESULTS = None
LAST_IN_MAPS = None


def _build(KC):
    """Build the Bass/Tile program for SK = KC*128 compacted+padded keys."""
    from contextlib import ExitStack

    import concourse.mybir as mybir
    import concourse.tile as tile
    from concourse import bacc

    SK = KC * 128
    f32 = mybir.dt.float32
    bf16 = mybir.dt.bfloat16
    i16 = mybir.dt.int16

    nc = bacc.Bacc(
        "TRN2",
        target_bir_lowering=False,
        debug=False,
        enable_asserts=False,
        num_devices=NCORES,
    )

    def din(name, shape, dt):
        return nc.dram_tensor(name, shape, dt, kind="ExternalInput").ap()

    d_xqT = din("xqT", [128, 4, SQ], bf16)
    d_xkT = din("xkT", [128, 4, SK], bf16)
    d_xvT = din("xvT", [128, 4, SK], bf16)
    d_bias = din("bias", [128, KC], f32)
    d_bias2 = din("bias2", [128, KC], f32)
    # wq/wk arrive as two separately-contiguous pieces: the oc0 block the
    # ramp needs (one fast 128KB transfer) and the oc1..3 remainder. Both
    # are fully contiguous in DRAM - strided small-segment DMAs measured
    # ~7us for 128KB vs ~1.7us contiguous.
    d_wq0 = din("wq0", [128, 4, 128], bf16)
    d_wqR = din("wqR", [128, 3, 4, 128], bf16)
    d_wk0 = din("wk0", [128, 4, 128], bf16)
    d_wkR = din("wkR", [128, 3, 4, 128], bf16)
    d_wvT = din("wvT", [128, 4, D], bf16)
    d_woT = din("woT", [128, 4, D], bf16)
    d_out = nc.dram_tensor("out", [SQ, D], f32, kind="ExternalOutput").ap()

    Exp = mybir.ActivationFunctionType.Exp
    mult = mybir.AluOpType.mult
    add = mybir.AluOpType.add

    def nslices(total, step=512):
        return [(s, min(step, total - s)) for s in range(0, total, step)]

    with tile.TileContext(nc) as tc:
        with ExitStack() as ctx:
            sb = ctx.enter_context(tc.tile_pool(name="sb", bufs=1))

            # ---- persistent SBUF tensors ----
            t_xqT = sb.tile([128, 4, SQ], bf16, tag="xqT")
            t_xkT = sb.tile([128, 4, SK], bf16, tag="xkT")
            t_xvT = sb.tile([128, 4, SK], bf16, tag="xvT")
            t_bias = sb.tile([128, KC], f32, tag="bias")
            t_bias2 = sb.tile([128, KC], f32, tag="bias2")
            t_wqT = sb.tile([128, 4, 4, 128], bf16, tag="wqT")
            t_wkT = sb.tile([128, 4, 4, 128], bf16, tag="wkT")
            t_wvT = sb.tile([128, 4, D], bf16, tag="wvT")
            t_woT = sb.tile([128, 4, D], bf16, tag="woT")
            t_QT = sb.tile([128, 4, SQ], bf16, tag="QT")
            t_KT = sb.tile([128, 4, SK], bf16, tag="KT")
            # Vpad[k, kc, h, 0:64] = V_h rows, [.., 64:128] = 1.0 (denominator)
            t_V = sb.tile([128, KC, H, 128], bf16, tag="V")
            # normalized attention out, head-PAIR packed: head 2c on
            # partitions 0:63, head 2c+1 on 64:127 (via DMA) -> K=128 final
            t_OnT = sb.tile([128, 4, SQ], bf16, tag="OnT")
            t_dumm = sb.tile([1, 8], f32, tag="dumm")
            t_dumo = sb.tile([1, 8], bf16, tag="dumo")

            # ---- DMA loads: the 3 issuing queues (sync/scalar HWDGE +
            # gpsimd SWDGE), scheduled by deadline. Full weight tensors go
            # first (contiguous 4KB/partition transfers are ~80 GB/s;
            # 128KB strided slices cost ~7us of fixed overhead), then the
            # x tensors in key chunks interleaved so every scores/PV
            # consumer's chunk lands before its emit slot.
            def dx(eng, t, d, sl):
                eng.dma_start(t[:, :, sl[0]:sl[0] + sl[1]],
                              d[:, :, sl[0]:sl[0] + sl[1]])

            # The 4 xq chunks are spread across all 3 queues: Q-proj(oc0)
            # needs all of xq before the first scores, so xq completion
            # (~11us) gates the whole exp stream.
            # xq gates everything (Q-proj(oc0) -> first scores -> exp
            # stream), so its 4 chunks lead all three queues; wvT moves
            # behind xq_ic3 on gpsimd (vproj is only needed ~10us later).
            nc.sync.dma_start(t_wqT[:, 0, :, :], d_wq0)
            nc.sync.dma_start(t_xqT[:, 0, :], d_xqT[:, 0, :])
            nc.sync.dma_start(t_xqT[:, 1, :], d_xqT[:, 1, :])
            nc.scalar.dma_start(t_bias[:], d_bias)
            nc.scalar.dma_start(t_wkT[:, 0, :, :], d_wk0)
            nc.scalar.dma_start(t_xqT[:, 2, :], d_xqT[:, 2, :])
            nc.gpsimd.dma_start(t_bias2[:], d_bias2)
            nc.gpsimd.dma_start(t_xqT[:, 3, :], d_xqT[:, 3, :])
            dx(nc.scalar, t_xkT, d_xkT, (0, 512))
            nc.gpsimd.dma_start(t_wvT[:], d_wvT)
            dx(nc.gpsimd, t_xvT, d_xvT, (0, 512))
            dx(nc.scalar, t_xkT, d_xkT, (512, 512))
            dx(nc.sync, t_xkT, d_xkT, (1024, 512))
            dx(nc.gpsimd, t_xvT, d_xvT, (512, 512))
            dx(nc.scalar, t_xkT, d_xkT, (2048, SK - 2048))
            dx(nc.sync, t_xvT, d_xvT, (1024, 512))
            dx(nc.sync, t_xkT, d_xkT, (1536, 512))
            nc.scalar.dma_start(t_wkT[:, 1:4, :, :], d_wkR)
            dx(nc.scalar, t_xvT, d_xvT, (1536, 512))
            dx(nc.gpsimd, t_xvT, d_xvT, (2048, SK - 2048))
            nc.sync.dma_start(t_wqT[:, 1:4, :, :], d_wqR)
            nc.sync.dma_start(t_woT[:], d_woT)

            # exp table preload (~2.7us) during the DMA ramp + V ones fill
            # on GpSimd so the DVE stays free for evacuations/exp tiles.
            nc.vector.memset(t_dumm[:], 0.0)
            nc.scalar.activation(t_dumo[:], t_dumm[:], Exp)
            nc.gpsimd.memset(t_V[:, :, :, 64:128], 1.0)
            t_warm = sb.tile([64, 512], bf16, tag="warm")
            nc.vector.memset(t_warm[:], 0.0)

            # PSUM budget (8 banks):
            #   pssa/pssb x1 buf = 4 banks (fp32 scores + projection psums)
            #   ppva/ppvb x1 buf = 4 banks (PV accumulators, fp32)
            ps_pool = ctx.enter_context(
                tc.tile_pool(name="ps_s", bufs=1, space="PSUM"))
            pv_pool = ctx.enter_context(
                tc.tile_pool(name="ps_pv", bufs=1, space="PSUM"))
            ep = ctx.enter_context(tc.tile_pool(name="ep", bufs=4))
            rp = ctx.enter_context(tc.tile_pool(name="rp", bufs=2))
            ob_pool = ctx.enter_context(tc.tile_pool(name="ob", bufs=4))

            _ptag = [0]

            def _proj_ps():
                _ptag[0] ^= 1
                return ps_pool.tile([128, 512], f32, name="psproj",
                                    tag="pssa" if _ptag[0] else "pssb")

            # ~3.8us of dummy matmuls at t~1us: the HAM clock gate needs
            # ~3.4us of sustained PE activity to lift the PE from 1.2 to
            # 2.4 GHz, so warm it up while the input DMAs are in flight.
            # Not more: these sit ahead of the projections on the in-order
            # PE queue, so extra warmup delays the first scores.
            for _ in range(6):
                ps = _proj_ps()
                nc.tensor.matmul(ps[:], t_warm[:, 0:128], t_warm[:],
                                 start=True, stop=True)

            # ---- projection units (emitted interleaved into attention) ----
            def qproj_unit(oc, qs, qn):
                ps = _proj_ps()
                for ic in range(4):
                    nc.tensor.matmul(
                        ps[:, :qn],
                        t_wqT[:, oc, ic, :],
                        t_xqT[:, ic, qs:qs + qn],
                        start=(ic == 0),
                        stop=(ic == 3),
                    )
                nc.vector.tensor_copy(t_QT[:, oc, qs:qs + qn], ps[:, :qn])

            def kproj_unit(oc, ks, kn):
                ps = _proj_ps()
                for ic in range(4):
                    nc.tensor.matmul(
                        ps[:, :kn],
                        t_wkT[:, oc, ic, :],
                        t_xkT[:, ic, ks:ks + kn],
                        start=(ic == 0),
                        stop=(ic == 3),
                    )
                nc.vector.tensor_copy(t_KT[:, oc, ks:ks + kn], ps[:, :kn])

            def vproj_unit(sc):
                ps = _proj_ps()
                for ic in range(4):
                    nc.tensor.matmul(
                        ps[:],
                        t_xvT[:, ic, sc * 128:(sc + 1) * 128],
                        t_wvT[:, ic, :],
                        start=(ic == 0),
                        stop=(ic == 3),
                    )
                nc.vector.tensor_copy(
                    t_V[:, sc, :, 0:64],
                    ps.rearrange("p (h d) -> p h d", h=H),
                )

            # pending projection units, popped between attention iterations.
            # Dependency-safe order; DMA chunk arrival comfortably leads the
            # schedule (exp stream consumes ~2.4us per kc iteration).
            from collections import deque
            pending = deque()

            def emit_pending(n):
                for _ in range(n):
                    if pending:
                        pending.popleft()()

            # ---- ramp: just enough projection for (hc=0, kc=0..3).
            # vproj(0/1) moves into the first iteration's slot so it does
            # not block the first scores on the xv DMA. ----
            for qs, qn in nslices(SQ):
                qproj_unit(0, qs, qn)
            kproj_unit(0, 0, 512)

            # remaining K-proj oc0 (keys 1024:SK), then oc1..3 QK, queued
            # for interleave. V-proj is scheduled explicitly two-ahead
            # inside the hc=0 loop (PV(0, kc) needs V[kc]). K-proj(0) rest
            # leads the queue, ordered so each unit's emit slot follows its
            # xk chunk's DMA arrival and precedes its consuming scores kc.
            k0_rest = nslices(SK)[1:]
            _k0_order = {512: 0, 2048: 1, 1024: 2, 1536: 3}
            k0_rest.sort(key=lambda x: _k0_order.get(x[0], 9))
            for ks, kn in k0_rest:
                pending.append(lambda ks=ks, kn=kn: kproj_unit(0, ks, kn))
            for oc in range(1, 4):
                for qs, qn in nslices(SQ):
                    pending.append(lambda oc=oc, qs=qs, qn=qn:
                                   qproj_unit(oc, qs, qn))
                for ks, kn in nslices(SK):
                    pending.append(lambda oc=oc, ks=ks, kn=kn:
                                   kproj_unit(oc, ks, kn))

            # ---- attention: 4 head pairs x KC key blocks ----
            for hc in range(4):
                ppv = {0: pv_pool.tile([128, SQ], f32, name="ppva", tag="ppva"),
                       1: pv_pool.tile([128, SQ], f32, name="ppvb", tag="ppvb")}

                def emit_pv(kc, es, hc=hc, ppv=ppv):
                    for hp in (0, 1):
                        for qs, qn in nslices(SQ):
                            nc.tensor.matmul(
                                ppv[hp][:, qs:qs + qn],
                                t_V[:, kc, 2 * hc + hp, :],
                                es[hp][:, qs:qs + qn],
                                start=(kc == 0),
                                stop=(kc == KC - 1),
                            )

                prev_es = prev_kc = None
                for kc in range(KC):
                    es = {}
                    # The two heads' scores matmuls contract over disjoint PE
                    # row halves (row_grp h0 / h64 via lhsT base partition).
                    # Emitting them hp-INTERLEAVED lets the PE run them
                    # concurrently per-subarray (row tiling: measured ~2x on
                    # K<=64 matmuls) instead of serializing same-row pairs.
                    pss = {hp: ps_pool.tile([128, SQ], f32, name="pss",
                                            tag="pssa" if hp == 0 else "pssb")
                           for hp in (0, 1)}
                    for qs, qn in nslices(SQ):
                        for hp in (0, 1):
                            nc.tensor.matmul(
                                pss[hp][:, qs:qs + qn],
                                t_KT[hp * 64:(hp + 1) * 64, hc,
                                     kc * 128:(kc + 1) * 128],
                                t_QT[hp * 64:(hp + 1) * 64, hc, qs:qs + qn],
                                start=True,
                                stop=True,
                            )
                    for hp in (0, 1):
                        if hp == 1 and _dve_kc(kc):
                            # Schraudolph exp on the DVE: int16 affine of the
                            # fp32 scores written into the bf16 tile's bytes
                            # (bitcast out AP), read back as bf16 by the PV
                            # matmul. Same SBUF as the ScalarE path.
                            e = ep.tile([128, SQ], bf16, name="e", tag="eb")
                            nc.vector.tensor_scalar(
                                e.bitcast(i16), pss[hp][:],
                                0.125 * SCH_A, t_bias2[:, kc:kc + 1],
                                op0=mult, op1=add,
                            )
                            es[hp] = e
                        else:
                            e = ep.tile([128, SQ], bf16, name="e",
                                        tag="ea" if hp == 0 else "eb")
                            nc.scalar.activation(
                                e[:], pss[hp][:], Exp,
                                bias=t_bias[:, kc:kc + 1], scale=0.125,
                            )
                            es[hp] = e
                    # PV is software-pipelined one kc behind the scores/exp:
                    # the PE consumes E tiles that finished during the
                    # previous iteration instead of blocking in-queue on the
                    # exp engines (measured ~127us of accumulated PE waits
                    # on ScalarE completions without the lag).
                    if prev_es is not None:
                        emit_pv(prev_kc, prev_es)
                    prev_es, prev_kc = es, kc
                    # Filler projection work AFTER this iteration's exp
                    # consumers are queued, so it overlaps them instead of
                    # delaying the next scores pair. V-proj runs two kc
                    # ahead of its PV consumer during hc0; the K-proj(0)
                    # remainder goes in fixed slots matched to its DMA
                    # arrival; other fillers take every other iteration.
                    if hc == 0:
                        if kc == 0:
                            vproj_unit(0)
                            vproj_unit(1)
                        if kc + 2 < KC:
                            vproj_unit(kc + 2)
                        if kc in (1, 5, 7, 9, 11, 13, 15):
                            emit_pending(1)
                    elif kc % 2 == 1:
                        emit_pending(1)
                emit_pv(prev_kc, prev_es)
                for hp in (0, 1):
                    # Evacuate num+denom PSUM to SBUF in one copy (frees the
                    # PV accumulator for the next pair ASAP). DVE lanes are
                    # partition-locked and the custom-DVE reciprocal only
                    # works at base partition 0, so DMA the denominator from
                    # partitions 64:127 down to 0:63 before inverting. The
                    # normalize multiply runs on GpSimd (idle engine) for
                    # hc<3 to keep the DVE free for exp tiles; the last
                    # chunk's chain is latency-critical (the output
                    # projection waits on it), so it runs on the faster DVE
                    # in q-halves to overlap with the ot DMA.
                    pv_sb = rp.tile([128, SQ], f32, tag="pvsb")
                    den_lo = rp.tile([64, SQ], f32, tag="denlo")
                    rc_lo = rp.tile([64, SQ], f32, tag="rcl")
                    if hc < 3:
                        nc.vector.tensor_copy(pv_sb[:], ppv[hp][:])
                        nc.sync.dma_start(den_lo[:], pv_sb[64:128, :])
                        nc.vector.reciprocal_approx_fast(rc_lo[:], den_lo[:])
                        if hp == 0:
                            nc.gpsimd.tensor_tensor(
                                t_OnT[0:64, hc, :], pv_sb[0:64, :],
                                rc_lo[:], mult)
                        else:
                            ot = rp.tile([64, SQ], bf16, tag="ottmp")
                            nc.gpsimd.tensor_tensor(
                                ot[:], pv_sb[0:64, :], rc_lo[:], mult)
                            nc.sync.dma_start(t_OnT[64:128, hc, :], ot[:])
                    else:
                        # Last chunk: the output projection waits on this
                        # chain, so pipeline it in q-halves (the den DMA
                        # latency hides behind the second copy) and use the
                        # idle gpsimd queue for the partition-move DMAs
                        # (sync is busy with output chunks).
                        ot = rp.tile([64, SQ], bf16, tag="ottmp")
                        for qs, qn in nslices(SQ):
                            nc.vector.tensor_copy(
                                pv_sb[:, qs:qs + qn], ppv[hp][:, qs:qs + qn])
                            nc.gpsimd.dma_start(den_lo[:, qs:qs + qn],
                                                pv_sb[64:128, qs:qs + qn])
                        for qs, qn in nslices(SQ):
                            nc.vector.reciprocal_approx_fast(
                                rc_lo[:, qs:qs + qn], den_lo[:, qs:qs + qn])
                            if hp == 0:
                                nc.vector.tensor_tensor(
                                    t_OnT[0:64, hc, qs:qs + qn],
                                    pv_sb[0:64, qs:qs + qn],
                                    rc_lo[:, qs:qs + qn], mult)
                            else:
                                nc.vector.tensor_tensor(
                                    ot[:, qs:qs + qn],
                                    pv_sb[0:64, qs:qs + qn],
                                    rc_lo[:, qs:qs + qn], mult)
                                nc.gpsimd.dma_start(
                                    t_OnT[64:128, hc, qs:qs + qn],
                                    ot[:, qs:qs + qn])

            emit_pending(len(pending))

            # ---- tail: output projection ----
            # The first 4 q-blocks' partial sums (head chunks 0..2) are
            # emitted right after the hc=3 evacuation so the PE computes
            # them during the ~6us evacuation chain (staying HAM-warm)
            # instead of idling; only the c=3 step waits for OnT[3].
            po_staged = []
            for qc in range(4):
                pool, tag = ((ps_pool, ("pssa", "pssb")[qc % 2]) if qc < 2
                             else (pv_pool, ("ppva", "ppvb")[qc % 2]))
                po = pool.tile([128, 512], f32, name="po", tag=tag)
                for c in range(3):
                    nc.tensor.matmul(
                        po[:],
                        t_OnT[:, c, qc * 128:(qc + 1) * 128],
                        t_woT[:, c, :],
                        start=(c == 0),
                        stop=False,
                    )
                po_staged.append(po)
            out_q = (nc.sync, nc.scalar, nc.gpsimd)
            for qc in range(SQ // 128):
                if qc < 4:
                    po = po_staged[qc]
                    nc.tensor.matmul(
                        po[:],
                        t_OnT[:, 3, qc * 128:(qc + 1) * 128],
                        t_woT[:, 3, :],
                        start=False,
                        stop=True,
                    )
                else:
                    po = _proj_ps()
                    for c in range(4):
                        nc.tensor.matmul(
                            po[:],
                            t_OnT[:, c, qc * 128:(qc + 1) * 128],
                            t_woT[:, c, :],
                            start=(c == 0),
                            stop=(c == 3),
                        )
                ob = ob_pool.tile([128, 512], f32, tag="ob")
                nc.vector.tensor_copy(ob[:], po[:])
                out_q[qc % 3].dma_start(d_out[qc * 128:(qc + 1) * 128, :],
                                        ob[:])

    nc.finalize()
    return nc


def _pack_T(x):
    """[n, 512] fp32 -> transposed bf16 packed [128, 4, n] (contiguous)."""
    n = x.shape[0]
    return np.ascontiguousarray(
        x.T.astype(BF16).reshape(4, 128, n).transpose(1, 0, 2)
    )


def _pack_W_oc(w):
    """[512, 512] W -> [128, oc, ic, 128] with W.T blocks: out[p, oc, ic, j]
    = W[oc*128+j, ic*128+p]. The oc slice is contiguous per partition."""
    return np.ascontiguousarray(
        np.asarray(w, np.float32).reshape(4, 128, 4, 128)
        .transpose(3, 0, 2, 1).astype(BF16)
    )


def kernel(query, key, value, mask, W_q, W_k, W_v, W_o):
    global LAST_RESULTS, LAST_IN_MAPS
    from concourse.bass_utils import run_bass_kernel_spmd

    query = np.asarray(query, np.float32)
    key = np.asarray(key, np.float32)
    value = np.asarray(value, np.float32)
    mask = np.asarray(mask)

    # -- host prep: mask compaction, transposes, bf16 casts, packing --
    sels = [np.nonzero(mask[b, 0, 0] != 0)[0] for b in range(B)]
    SK = ((max(len(s) for s in sels) + 127) // 128) * 128
    KC = SK // 128

    per_batch = []
    for b in range(B):
        sel = sels[b]
        nk = len(sel)
        xk = np.zeros((SK, D), np.float32)
        xk[:nk] = key[b][sel]
        xv = np.zeros((SK, D), np.float32)
        xv[:nk] = value[b][sel]
        bias = np.full(SK, -30.0, np.float32)
        bias[:nk] = 0.0
        bias_col = np.ascontiguousarray(bias.reshape(KC, 128).T)
        per_batch.append({
            "xkT": _pack_T(xk),
            "xvT": _pack_T(xv),
            "bias": bias_col,
            "bias2": np.ascontiguousarray(
                SCH_A * bias_col + np.float32(SCH_B)),
        })

    wq_oc = _pack_W_oc(W_q)                      # [128, oc, ic, 128]
    wk_oc = _pack_W_oc(W_k)
    wq0 = np.ascontiguousarray(wq_oc[:, 0])
    wqR = np.ascontiguousarray(wq_oc[:, 1:4])
    wk0 = np.ascontiguousarray(wk_oc[:, 0])
    wkR = np.ascontiguousarray(wk_oc[:, 1:4])
    wvT = _pack_T(np.asarray(W_v, np.float32))
    woT = _pack_T(np.asarray(W_o, np.float32))  # [128, 4, 512], head-pair rows

    in_maps = []
    for c in range(NCORES):
        b, qc = divmod(c, QSH)
        xq = query[b, qc * SQ:(qc + 1) * SQ]
        in_maps.append({
            "xqT": _pack_T(xq),
            "xkT": per_batch[b]["xkT"],
            "xvT": per_batch[b]["xvT"],
            "bias": per_batch[b]["bias"],
            "bias2": per_batch[b]["bias2"],
            "wq0": wq0, "wqR": wqR, "wk0": wk0, "wkR": wkR,
            "wvT": wvT, "woT": woT,
        })

    if KC not in _BUILD_CACHE:
        _BUILD_CACHE[KC] = _build(KC)
    nc = _BUILD_CACHE[KC]

    LAST_IN_MAPS = in_maps
    res = run_bass_kernel_spmd(nc, in_maps, core_ids=list(range(NCORES)))
    LAST_RESULTS = res

    out = np.empty((B, S, D), np.float32)
    for c in range(NCORES):
        b, qc = divmod(c, QSH)
        out[b, qc * SQ:(qc + 1) * SQ] = res.results[c]["out"]
    return out

